# revision 2
# baseline (speedup 1.0000x reference)
"""Chamfer deviation L2 kernel for Trainium2 — v4 (single-produce, block-local
masks with winner-weighted counts).

Contract: kernel(xyz1, xyz2) takes FULL inputs [8, 4096, 3] fp32 and returns
the FULL output [4] fp32 (cd_l2 scalar + 3-vector mean deviation).

Per core = one batch. x[n, m] = -d[n, m] = 2*x1.x2 - |x1|^2 - |x2|^2 on the PE
via a 5-term augmented contraction in float32r (single-pass fp32 matmul:
1 cycle/column), 3-way row-packed at PE row-groups 0/32/64.

Key idea vs earlier versions: the distance matrix is produced ONCE. Each
produced block [128, 1536|1024] is consumed by BOTH scans immediately:
  - DVE tensor_reduce (negate) -> ngb[p, t*3+b] = -blockmax (block-local).
  - ACT Sign(x + ngb_b) -> scr_b in {0 at the row's BLOCK-argmax, -1 else},
    biased by the block-local max, so the sign needs no cross-block result
    and the PSUM tile is freed right after — no recompute, no residency wall.
GPSIMD (otherwise idle) combines the 3 block maxes per tile into the row max
(mn[p,t] = min_b ngb = dmin) and winner flags w_b[p] = (ngb_b == mn) in bf16.
PE count matmuls use w_b as the stationary vector: only rows whose TRUE
argmin lies in block b contribute, so each row adds one 0-entry (at its
global argmin) and -1s across its winning block. Accumulated per chunk-col
group into 2 persistent PSUM acc banks over all 32 tiles, drained once per
orientation. Host recovers counts: within each block's col-range,
c[m] = acc[m] + C_b with C_b = sum_range(acc) / (1 - block_width).

PSUM: work 2x[128,1536] (6 banks) + acc 2x[128,512] (2) = 8.

Host: dist sums from mn (=dmin); V1 = sum_m c1[m]*xyz2[m]; final in float64.
"""

import sys

sys.path.insert(0, "/opt/trn_rl_repo")

import numpy as np

import concourse.bass as bass
import concourse.bacc as bacc
import concourse.tile as tile
from concourse import mybir
from concourse.bass_utils import run_bass_kernel_spmd

F32 = mybir.dt.float32
F32R = mybir.dt.float32r
BF16 = mybir.dt.bfloat16
AX = mybir.AxisListType
OP = mybir.AluOpType
AF = mybir.ActivationFunctionType

B, N, M = 8, 4096, 4096
P = 128
LAG_CNT = 4      # counts lag det by BATCH_T tiles (winner flags batched)
BATCH_T = 4      # tiles per batched row-min/winner-flag DVE op

# blocks of the 4096-col row: (col_offset, width, n_packed_chunks)
BLOCKS = [(0, 1024, 2), (1024, 1024, 2), (2048, 1024, 2), (3072, 1024, 2)]
NCH = len(BLOCKS)
SLOT_W = 1024    # PSUM work slot width; pool = WORK_BUFS x 2 banks
WORK_BUFS = 3    # 3x[1024] work (6 banks) + 2x[512] acc = 8 banks


def build_nc(n=N, m=M, loop=1, f32r=True):
    """Build the per-core Bacc program (SPMD: same program on all 8 cores)."""
    assert n == m
    nt = n // P              # 32 tile-rows per orientation
    MMDT = F32R if f32r else F32

    nc = bacc.Bacc("TRN2", target_bir_lowering=False, debug=False)

    # weights replicated at partition rows 0-4, 32-36, 64-68
    d_wA = nc.dram_tensor("wrepA", [96, n], MMDT, kind="ExternalInput")
    d_wB = nc.dram_tensor("wrepB", [96, n], MMDT, kind="ExternalInput")
    # packed streams: block bi's chunk g at partitions 32g..32g+4
    d_sA = nc.dram_tensor("strmA", [96, 512 * NCH], MMDT, kind="ExternalInput")
    d_sB = nc.dram_tensor("strmB", [96, 512 * NCH], MMDT, kind="ExternalInput")

    d_mn1 = nc.dram_tensor("mn1", [P, nt], F32, kind="ExternalOutput")
    d_mn2 = nc.dram_tensor("mn2", [P, nt], F32, kind="ExternalOutput")
    d_c1p = nc.dram_tensor("c1p", [1, m], F32, kind="ExternalOutput")
    d_c2p = nc.dram_tensor("c2p", [1, n], F32, kind="ExternalOutput")

    with tile.TileContext(nc) as tc:
        from contextlib import ExitStack

        with ExitStack() as ctx:
            cpool = ctx.enter_context(tc.tile_pool(name="const", bufs=1))
            work_ps = ctx.enter_context(
                tc.tile_pool(name="workps", bufs=WORK_BUFS, space="PSUM")
            )
            acc_ps = ctx.enter_context(tc.tile_pool(name="accps", bufs=2, space="PSUM"))
            scr_pool = ctx.enter_context(tc.tile_pool(name="scr", bufs=24))
            sacc_pool = ctx.enter_context(tc.tile_pool(name="sacc", bufs=4))

            wA = cpool.tile([96, n], MMDT, tag="wA")
            wB = cpool.tile([96, n], MMDT, tag="wB")
            sA = cpool.tile([96, 512 * NCH], MMDT, tag="sA")
            sB = cpool.tile([96, 512 * NCH], MMDT, tag="sB")
            mn1_sb = cpool.tile([P, nt], F32, tag="mn1sb")
            mn2_sb = cpool.tile([P, nt], F32, tag="mn2sb")
            ngbA = cpool.tile([P, nt * NCH], F32, tag="ngbA")
            ngbB = cpool.tile([P, nt * NCH], F32, tag="ngbB")
            wgtA = cpool.tile([P, nt * NCH], BF16, tag="wgtA")
            wgtB = cpool.tile([P, nt * NCH], BF16, tag="wgtB")
            tmpA = cpool.tile([P, nt], F32, tag="tmpA")
            tmpB = cpool.tile([P, nt], F32, tag="tmpB")

            nc.sync.dma_start(wA[:, :], d_wA.ap())
            nc.sync.dma_start(wB[:, :], d_wB.ap())
            nc.sync.dma_start(sA[:, :], d_sA.ap())
            nc.sync.dma_start(sB[:, :], d_sB.ap())

            def produce(pt, w, s, t, bi):
                """Produce x for tile-row t, block bi into psum tile pt.
                npk row-packed K=5 matmuls share one streamed 512-col block."""
                off, wd, npk = BLOCKS[bi]
                blk = s[:, bi * 512:(bi + 1) * 512]
                for g in range(npk):
                    nc.tensor.matmul(
                        pt[:, g * 512:(g + 1) * 512],
                        lhsT=w[32 * g:32 * g + 5, t * P:(t + 1) * P],
                        rhs=blk[32 * g:32 * g + 5, :],
                        start=True,
                        stop=True,
                        tile_position=(32 * g, 0),
                    )

            def detmask_tile(w, s, ngb, wgt, mn_sb, tmp, scr_map, t):
                for bi in range(NCH):
                    wd = BLOCKS[bi][1]
                    ptf = work_ps.tile([P, SLOT_W], F32, tag="workps", name="ptf")
                    pt = ptf[:, 0:wd]
                    produce(pt, w, s, t, bi)
                    nc.vector.tensor_reduce(
                        ngb[:, t * NCH + bi:t * NCH + bi + 1],
                        pt[:, :],
                        axis=AX.X,
                        op=OP.max,
                        negate=True,
                    )
                    scrf = scr_pool.tile([P, SLOT_W], BF16, tag="scr", name="scrf")
                    scr = scrf[:, 0:wd]
                    nc.scalar.activation(
                        scr[:, :],
                        pt[:, :],
                        AF.Sign,
                        bias=ngb[:, t * NCH + bi:t * NCH + bi + 1],
                        scale=1.0,
                    )
                    scr_map[(t, bi)] = scr
                # DVE (batched every BATCH_T tiles): row min of the block
                # ngb values (= dmin) and winner flags w_b = (ngb_b <= mn).
                if t % BATCH_T == BATCH_T - 1:
                    t0 = t - (BATCH_T - 1)
                    o = t0 * NCH
                    span = BATCH_T * NCH
                    nc.vector.tensor_reduce(
                        mn_sb[:, t0:t + 1],
                        ngb[:, o:o + span].rearrange("p (a b) -> p a b", b=NCH),
                        axis=AX.X,
                        op=OP.min,
                    )
                    nc.vector.tensor_tensor(
                        wgt[:, o:o + span],
                        ngb[:, o:o + span],
                        mn_sb[:, t0:t + 1]
                        .unsqueeze(2)
                        .to_broadcast([P, BATCH_T, NCH]),
                        op=OP.is_le,
                    )

            def count_tile(wgt, scr_map, acc_map, d_cp, t):
                if t == 0:
                    acc_map[0] = acc_ps.tile([P, 512], F32, tag="accps", name="acc0")
                    acc_map[1] = acc_ps.tile([P, 512], F32, tag="accps", name="acc1")
                cum = [0]
                for _, wd, npk in BLOCKS:
                    cum.append(cum[-1] + npk)
                for c in range(n // 512):          # global chunk index
                    bi = next(i for i in range(NCH) if cum[i + 1] > c)
                    g = c - cum[bi]
                    scr = scr_map[(t, bi)]
                    h, j = c // 4, c % 4
                    nc.tensor.matmul(
                        acc_map[h][32 * j:32 * j + 1, :],
                        lhsT=wgt[:, t * NCH + bi:t * NCH + bi + 1],
                        rhs=scr[:, g * 512:(g + 1) * 512],
                        start=(t == 0),
                        stop=(t == nt - 1),
                        tile_position=(0, 32 * j),
                    )
                if t == nt - 1:
                    for h in range(2):
                        acc = acc_map.pop(h)
                        sacc = sacc_pool.tile([1, 2048], F32, tag="sacc", name="sacc")
                        for j in range(4):
                            nc.scalar.copy(
                                sacc[0:1, 512 * j:512 * (j + 1)],
                                acc[32 * j:32 * j + 1, 0:512],
                            )
                        nc.sync.dma_start(
                            d_cp.ap()[0:1, 2048 * h:2048 * (h + 1)], sacc[:, :]
                        )

            def orientation(w, s, ngb, wgt, mn_sb, tmp, d_mn, d_cp):
                scr_map, acc_map = {}, {}
                for step in range(nt + LAG_CNT):
                    if step < nt:
                        detmask_tile(w, s, ngb, wgt, mn_sb, tmp, scr_map, step)
                    tcnt = step - LAG_CNT
                    if 0 <= tcnt < nt:
                        count_tile(wgt, scr_map, acc_map, d_cp, tcnt)
                        for bi in range(NCH):
                            scr_map.pop((tcnt, bi))
                nc.sync.dma_start(d_mn.ap(), mn_sb[:, :])

            def body():
                orientation(wA, sA, ngbA, wgtA, mn1_sb, tmpA, d_mn1, d_c1p)
                orientation(wB, sB, ngbB, wgtB, mn2_sb, tmpB, d_mn2, d_c2p)

            if loop > 1:
                with tc.For_i(0, loop, 1):
                    body()
            else:
                body()

    nc.compile()
    return nc


def _augment(xyz, n):
    """[n,3] -> (lhs_aug [5,n] weights-side, rhs_aug [5,n] stream-side)."""
    x, y, z = xyz[:, 0].copy(), xyz[:, 1].copy(), xyz[:, 2].copy()
    sq = (x * x + y * y) + z * z
    one = np.ones(n, np.float32)
    lhs = np.stack([2 * x, 2 * y, 2 * z, -sq, -one]).astype(np.float32)
    rhs = np.stack([x, y, z, one, sq]).astype(np.float32)
    return lhs, rhs


def make_inputs(xyz1b, xyz2b, n=N, m=M):
    """Build packed augmented operands for one batch."""
    assert n == m
    lhs1, rhs1 = _augment(xyz1b, n)
    lhs2, rhs2 = _augment(xyz2b, m)

    def wrep(lhs):
        w = np.zeros((96, n), np.float32)
        w[0:5] = lhs
        w[32:37] = lhs
        w[64:69] = lhs
        return w

    def spack(rhs):
        s = np.zeros((96, 512 * NCH), np.float32)
        for bi, (off, wd, npk) in enumerate(BLOCKS):
            for g in range(npk):
                s[32 * g:32 * g + 5, bi * 512:(bi + 1) * 512] = rhs[
                    :, off + g * 512:off + (g + 1) * 512
                ]
        return s

    return {
        "wrepA": wrep(lhs1),
        "strmA": spack(rhs2),
        "wrepB": wrep(lhs2),
        "strmB": spack(rhs1),
    }


def decode_core(out, xyz1b, xyz2b, n=N, m=M, verbose=False):
    """Decode one core's outputs into partial sums (float64)."""
    mn1 = out["mn1"].astype(np.float64)   # [128, nt]; mn = -rowmax = dmin
    mn2 = out["mn2"].astype(np.float64)
    dist1 = mn1.T.reshape(n)
    dist2 = mn2.T.reshape(m)

    def counts(cp):
        # within each block's col-range: acc[m] = c[m] - C_b where C_b is the
        # number of rows whose argmin falls in block b (over all 32 tiles);
        # sum_range(acc) = C_b - width*C_b  =>  C_b = sum_range / (1 - width)
        acc = cp.astype(np.float64).reshape(-1)
        c = np.empty_like(acc)
        for off, wd, npk in BLOCKS:
            a = acc[off:off + wd]
            C_b = a.sum() / (1.0 - wd)
            c[off:off + wd] = a + C_b
        return c

    c1 = counts(out["c1p"])
    c2 = counts(out["c2p"])
    if verbose:
        print(
            f"  count sums: c1={c1.sum():.1f} (want {n}), c2={c2.sum():.1f} (want {m})"
        )

    V1 = c1 @ xyz2b.astype(np.float64)
    V2 = c2 @ xyz1b.astype(np.float64)
    return dist1.sum(), dist2.sum(), V1, V2, c1.sum(), c2.sum()


_NC_CACHE = {}
LAST_RESULTS = None


def kernel(xyz1, xyz2, trace=False, verbose=False):
    global LAST_RESULTS
    xyz1 = np.asarray(xyz1, dtype=np.float32)
    xyz2 = np.asarray(xyz2, dtype=np.float32)
    b, n, _ = xyz1.shape
    m = xyz2.shape[1]

    key = (n, m)
    if key not in _NC_CACHE:
        _NC_CACHE[key] = build_nc(n, m)
    nc = _NC_CACHE[key]

    in_maps = [make_inputs(xyz1[i], xyz2[i], n, m) for i in range(b)]
    res = run_bass_kernel_spmd(nc, in_maps, core_ids=list(range(b)), trace=trace)
    LAST_RESULTS = res

    S1 = S2 = 0.0
    V1 = np.zeros(3)
    V2 = np.zeros(3)
    for i in range(b):
        s1, s2, v1, v2, c1s, c2s = decode_core(
            res.results[i], xyz1[i], xyz2[i], n, m, verbose=verbose
        )
        if abs(c1s - n) > 16 or abs(c2s - m) > 16:
            print(
                f"kernel: warning core {i}: count sums c1={c1s:.1f}/{n} "
                f"c2={c2s:.1f}/{m}"
            )
        S1 += s1
        S2 += s2
        V1 += v1
        V2 += v2

    sum1 = xyz1.astype(np.float64).sum(axis=(0, 1))
    sum2 = xyz2.astype(np.float64).sum(axis=(0, 1))
    cd_l2 = S1 / (b * n) + S2 / (b * m)
    cd_dev = (sum1 - V1) / (b * n) + (sum2 - V2) / (b * m)
    return np.concatenate([[cd_l2], cd_dev]).astype(np.float32)
